# revision 18
# baseline (speedup 1.0000x reference)
"""Trainium2 Bass kernel for the DyadBlock problem.

Math (reference):
    xb   = x.reshape(DY, DI, B)
    incl = cumsum(xb, axis=0)             # inclusive prefix over dyads
    total= incl[-1]
    out[d] = w_lower[d] @ incl[d] + w_upper[d] @ (total - incl[d]) + bias

Rewrite (wd = w_lower - w_upper):
    out[d] = wd[d] @ incl[d] + w_upper[d] @ total + bias

v4 design (pure data parallel over batch, 8 cores; each core owns a
1024-column batch slice processed as 2 x 512-column chunks).  Dyads are
grouped 4 per 128-partition tile (64 tiles of 4 dyads); tiles are split
into NQ=4 chains (quarters) of ql=16 steps.  The cumsum is never
materialized -- instead, per tile t (chain q, step s):

    out_t = W1_t.T @ X_t                (block-lower-triangular wd blocks:
                                         within-tile prefix, fused m1+m3)
          + WDROW_{s}.T @ R_{q,s}       (carry: R = running sum of the
                                         chain's previous tiles, K=32 at
                                         row strip q -> the 4 chains' carry
                                         matmuls hit 4 distinct PE row
                                         groups and run CONCURRENTLY)
          + WQ_t.T @ Estack             (wu@total + inter-quarter fixup;
                                         Estack strip p = quarter-p total)
          + bias

Phase A only accumulates R: per (chunk, step) 4 col-tiled K=128->M=32
matmuls (concurrent) add each chain's tile-sum into one persistent PSUM
accumulator; a DVE/ACT snapshot before each step gives R_{q,s}, and the
final snapshot IS Estack.  This kills the 128 incl PSUM->SBUF copies and
all m1/m2 passes of the previous design (PE ~120us -> ~77us, DVE+ACT
~90us -> ~56us each).

x is loaded HALF-WIDTH per chunk (chunk 0 fully first) so phase B of
chunk 0 (and its output stores) overlaps the x load of chunk 1.  x stays
on a solo SWDGE queue (concurrent bulk streams thrash DRAM pages);
outputs alternate over the two HWDGE rings; w1/wq stream on the ACT ring.
fp16 operands throughout (1 cycle/row on the PE, ample precision here).
"""

import os

import numpy as np

import concourse.bacc as bacc
import concourse.mybir as mybir
import concourse.tile as tile
from concourse import bass_utils
from concourse.tile_rust import add_dep_helper

DY, DO, DI = 256, 32, 32
B = 8192
NCORES = 8
BC = B // NCORES  # batch columns per core
NQ = 4            # carry chains (quarters) per chunk

_cache = {}
last_results = None


def _cfg():
    mm = os.environ.get("DYAD_MM_DT", "fp16")
    mm_dt = {
        "f32": mybir.dt.float32,
        "f32r": mybir.dt.float32r,
        "bf16": mybir.dt.bfloat16,
        "fp16": mybir.dt.float16,
    }[mm]
    out_s = os.environ.get("DYAD_OUT_DT", "same")
    out_dt = {"f32": mybir.dt.float32, "same": mm_dt}[out_s]
    return mm_dt, out_dt


def build(mm_dt, out_dt, bc=BC, n=512, nt=DY // 4, o_bufs=6, po_bufs=5):
    """Build + compile the per-core Bass program."""
    f32 = mybir.dt.float32
    nchunk = bc // n
    ql = nt // NQ  # tiles (steps) per carry chain
    nc = bacc.Bacc("TRN2", target_bir_lowering=False, debug=False,
                   num_devices=NCORES)
    x_d = nc.dram_tensor("x", [nt * 128, bc], mm_dt, kind="ExternalInput").ap()
    w1_d = nc.dram_tensor("w1", [128, nt * 128], mm_dt,
                          kind="ExternalInput").ap()
    wq_d = nc.dram_tensor("wq", [128, nt * 128], mm_dt,
                          kind="ExternalInput").ap()
    wdrow_d = nc.dram_tensor("wdrow", [128, ql * 128], mm_dt,
                             kind="ExternalInput").ap()
    s4_d = nc.dram_tensor("s4", [128, 32], mm_dt, kind="ExternalInput").ap()
    bias_d = nc.dram_tensor("biast", [128, nt], f32, kind="ExternalInput").ap()
    out_d = nc.dram_tensor("out", [nt * 128, bc], out_dt,
                           kind="ExternalOutput").ap()

    with tile.TileContext(nc) as tc:
        with tc.tile_pool(name="wpool", bufs=1) as wp, \
             tc.tile_pool(name="xpool", bufs=nchunk * nt) as xp, \
             tc.tile_pool(name="opool", bufs=o_bufs) as op, \
             tc.tile_pool(name="rpool", bufs=nchunk * (ql - 1)) as rp, \
             tc.tile_pool(name="epool", bufs=nchunk) as ep, \
             tc.tile_pool(name="rps_pool", bufs=nchunk, space="PSUM") as pip, \
             tc.tile_pool(name="pout_pool", bufs=po_bufs, space="PSUM") as pop:
            w1 = wp.tile([128, nt * 128], mm_dt)
            wq = wp.tile([128, nt * 128], mm_dt)
            wdrow = wp.tile([128, ql * 128], mm_dt)
            s4 = wp.tile([128, 32], mm_dt)
            biast = wp.tile([128, nt], f32)
            # PE warmup: the HAM clock gate starts at K=4/8 (1.2 GHz) and
            # only reaches 2.4 GHz after ~3.4us of sustained activity; a
            # burst of dependency-free matmuls starts that clock while the
            # first x tiles are still in flight.
            warm_n = int(os.environ.get("DYAD_WARM", "10"))
            if warm_n:
                wrm = wp.tile([128, 128], mm_dt)
                nc.vector.memset(wrm[:], 0.0)
                wps = pop.tile([128, n], f32, tag="pout", name="warm")
                for i in range(warm_n):
                    nc.tensor.matmul(wps[:, 0:128], wrm[:], wrm[:],
                                     start=(i == 0), stop=(i == warm_n - 1))
            # small consts on the SP ring, bulk weights on the ACT ring
            # (both HWDGE); x has the SWDGE queue to itself.
            nc.sync.dma_start(out=s4[:], in_=s4_d)
            nc.sync.dma_start(out=biast[:], in_=bias_d)
            nc.sync.dma_start(out=wdrow[:], in_=wdrow_d)
            nc.scalar.dma_start(out=w1[:], in_=w1_d)
            nc.scalar.dma_start(out=wq[:], in_=wq_d)

            # ---- x loads: half-width, chunk 0 first, chain-interleaved ----
            xts = [[None] * nt for _ in range(nchunk)]
            issue_order = [q * ql + s for s in range(ql) for q in range(NQ)]
            for c in range(nchunk):
                for tt in issue_order:
                    xt = xp.tile([128, n], mm_dt, tag="x", name=f"x_{c}_{tt}")
                    nc.gpsimd.dma_start(
                        out=xt[:],
                        in_=x_d[128 * tt:128 * (tt + 1), c * n:(c + 1) * n])
                    xts[c][tt] = xt

            # ---- phase A: R-chain accumulation (per chunk) ----
            # Rps strip q accumulates chain q's running tile-sums; the
            # snapshot before step s is the carry operand R_{q,s}, and the
            # final snapshot is Estack (strip p = quarter-p total).
            rhist = [[None] * ql for _ in range(nchunk)]
            estack = [None] * nchunk
            rflip = 0
            for c in range(nchunk):
                rps = pip.tile([128, n], f32, tag="rps", name=f"rps_{c}")
                for s in range(ql):
                    if s > 0:
                        rh = rp.tile([128, n], mm_dt, tag="rh",
                                     name=f"rh_{c}_{s}")
                        if rflip == 0:
                            nc.vector.tensor_copy(out=rh[:], in_=rps[:])
                        else:
                            nc.scalar.copy(out=rh[:], in_=rps[:])
                        rflip ^= 1
                        rhist[c][s] = rh
                    for q in range(NQ):
                        tt = q * ql + s
                        nc.tensor.matmul(
                            rps[32 * q:32 * (q + 1), :], s4[:],
                            xts[c][tt][:],
                            start=(s == 0), stop=True,
                            tile_position=(0, 32 * q))
                est = ep.tile([128, n], mm_dt, tag="est", name=f"est_{c}")
                if rflip == 0:
                    nc.vector.tensor_copy(out=est[:], in_=rps[:])
                else:
                    nc.scalar.copy(out=est[:], in_=rps[:])
                rflip ^= 1
                estack[c] = est

            # ---- phase B: outputs (per chunk, 4 chains per step) ----
            eflip = 0
            oflip = 0
            for c in range(nchunk):
                for s in range(ql):
                    pouts, lastmm = [], []
                    for q in range(NQ):
                        tt = q * ql + s
                        pout = pop.tile([128, n], f32, tag="pout",
                                        name=f"pout_{c}_{tt}")
                        m = nc.tensor.matmul(
                            pout[:], w1[:, 128 * tt:128 * (tt + 1)],
                            xts[c][tt][:], start=True, stop=False,
                            tile_position=(0, 0))
                        pouts.append(pout)
                        lastmm.append(m)
                    if s > 0:
                        # the 4 chains' carry matmuls sit at 4 distinct row
                        # groups -> concurrent on the PE
                        for q in range(NQ):
                            m = nc.tensor.matmul(
                                pouts[q][:],
                                wdrow[32 * q:32 * (q + 1),
                                      128 * s:128 * (s + 1)],
                                rhist[c][s][32 * q:32 * (q + 1), :],
                                start=False, stop=False,
                                tile_position=(32 * q, 0))
                            add_dep_helper(m.ins, lastmm[q].ins, sync=False,
                                           reason="psum order w1->wdrow")
                            lastmm[q] = m
                    for q in range(NQ):
                        tt = q * ql + s
                        m = nc.tensor.matmul(
                            pouts[q][:], wq[:, 128 * tt:128 * (tt + 1)],
                            estack[c][:], start=False, stop=True,
                            tile_position=(0, 0))
                        add_dep_helper(m.ins, lastmm[q].ins, sync=False,
                                       reason="psum order ->wq")
                    for q in range(NQ):
                        tt = q * ql + s
                        outt = op.tile([128, n], out_dt, tag="out",
                                       name=f"out_{c}_{tt}")
                        if eflip == 0:
                            nc.scalar.add(out=outt[:], in_=pouts[q][:],
                                          add=biast[:, tt:tt + 1])
                        else:
                            nc.vector.tensor_scalar_add(
                                out=outt[:], in0=pouts[q][:],
                                scalar1=biast[:, tt:tt + 1])
                        eflip ^= 1
                        # alternate output stores over the two HWDGE rings:
                        # one ring sustained only ~220 GB/s on these writes.
                        oeng = (nc.scalar, nc.sync)[oflip]
                        oflip ^= 1
                        oeng.dma_start(
                            out=out_d[128 * tt:128 * (tt + 1),
                                      c * n:(c + 1) * n],
                            in_=outt[:])
    nc.compile()
    return nc


def host_weights(w_upper, w_lower, bias, np_io, nt=DY // 4):
    """Host-side weight layouts (lhsT conventions, see build())."""
    w_upper = np.asarray(w_upper, dtype=np.float32)
    w_lower = np.asarray(w_lower, dtype=np.float32)
    bias = np.asarray(bias, dtype=np.float32)
    ql = nt // NQ
    wd = w_lower - w_upper
    wdT = wd.transpose(0, 2, 1)    # [d, i, j] = wd[d][j, i]
    wuT = w_upper.transpose(0, 2, 1)
    wdr = wdT.reshape(nt, 4, 32, 32)   # [t, a, i, j]
    wur = wuT.reshape(nt, 4, 32, 32)

    # W1: block (b, a) of tile t = wd[4t+a].T for b <= a (within-tile
    # inclusive prefix times wd), as [128, nt*128]
    W1 = np.zeros((4, 32, nt, 4, 32), np.float32)   # [b, i, t, a, j]
    for a in range(4):
        for b in range(a + 1):
            W1[b, :, :, a, :] = wdr[:, a].transpose(1, 0, 2)
    W1 = np.ascontiguousarray(W1.reshape(128, nt * 128))

    # WDROW: [32q+i, 128s+32a+j] = wd[4(q*ql+s)+a][j, i]
    WDROW = np.zeros((4, 32, ql, 4, 32), np.float32)   # [q, i, s, a, j]
    for q in range(NQ):
        for s in range(ql):
            WDROW[q, :, s, :, :] = wdr[q * ql + s].transpose(1, 0, 2)
    WDROW = np.ascontiguousarray(WDROW.reshape(128, ql * 128))

    # WQ: rows 32p+i, cols 128t+32a+j = (wu + (quarter(t) > p) wd)[4t+a][j,i]
    quarter = (np.arange(nt) // ql)
    W = np.zeros((4, 32, nt, 4, 32), np.float32)
    for p in range(NQ):
        blk = wur + ((quarter > p).astype(np.float32))[:, None, None, None] * wdr
        W[p] = blk.transpose(2, 0, 1, 3)   # [i, t, a, j]
    WQ = np.ascontiguousarray(W.reshape(128, nt * 128))

    S4 = np.tile(np.eye(32, dtype=np.float32), (4, 1))
    BIAST = np.ascontiguousarray(
        bias.reshape(nt, 4, 32).transpose(1, 2, 0).reshape(128, nt))
    return {
        "w1": W1.astype(np_io, copy=False),
        "wq": WQ.astype(np_io, copy=False),
        "wdrow": WDROW.astype(np_io, copy=False),
        "s4": S4.astype(np_io, copy=False),
        "biast": BIAST,
    }


def _run_profiled(nc, in_maps):
    """Mirror of bass_utils' axon trace branch; the antenv.axon_hooks
    module is absent in this image, so drive the ctypes NTFF hook from
    trn_agent_boot directly and post-process with bass_utils helpers."""
    import glob
    import tempfile

    import gauge.profiler
    from concourse import bass2jax
    from concourse._compat import FishPath
    from trn_agent_boot.trn_boot import _ntff_profile_via_ctypes

    hook = _ntff_profile_via_ctypes("/opt/axon/libaxon_pjrt.so")
    if hook is None:
        raise RuntimeError("no NTFF profile symbols in libaxon_pjrt.so")
    neff_dir = tempfile.mkdtemp(prefix="dyad_prof_")
    with hook(neff_dir, [0]):
        results = bass2jax.run_bass_via_pjrt(nc, in_maps, n_cores=NCORES)
    ntffs = glob.glob(os.path.join(neff_dir, "*_body*.ntff"))
    if not ntffs:
        raise RuntimeError(f"no NTFFs in {neff_dir}")
    profile = gauge.profiler.Profile(
        profile_path=FishPath(neff_dir),
        kernel_dev_mode=True,
        profile_on_exit=False,
        bass_kernel=nc.m,
        offline_processing=True,
        fname="*_body*",
        metadata={},
    )
    return bass_utils._process_ntff_profile(
        profile, neff_dir, nc, list(range(NCORES)), [0], False, {},
        trace_events=False,
    ).as_bass_kernel_results(results)


def kernel(x, w_upper, w_lower, bias):
    global last_results
    mm_dt, out_dt = _cfg()
    key = (mm_dt, out_dt)
    if key not in _cache:
        _cache[key] = build(mm_dt, out_dt)
    nc = _cache[key]

    np_io = mybir.dt.np(mm_dt)
    x = np.asarray(x, dtype=np.float32)
    w = host_weights(w_upper, w_lower, bias, np_io)
    in_maps = []
    for cidx in range(NCORES):
        xs = np.ascontiguousarray(x[:, cidx * BC:(cidx + 1) * BC]).astype(
            np_io, copy=False)
        in_maps.append({"x": xs, **w})

    if os.environ.get("DYAD_TRACE", "0") == "1":
        try:
            res = _run_profiled(nc, in_maps)
        except Exception as e:  # profiling is best-effort
            print("profiled run failed (%s); falling back" % e)
            res = bass_utils.run_bass_kernel_spmd(
                nc, in_maps, core_ids=list(range(NCORES)), trace=False)
    else:
        res = bass_utils.run_bass_kernel_spmd(
            nc, in_maps, core_ids=list(range(NCORES)), trace=False)
    last_results = res
    out = np.concatenate([res.results[c]["out"] for c in range(NCORES)], axis=1)
    return np.ascontiguousarray(out, dtype=np.float32)
